# revision 1
# baseline (speedup 1.0000x reference)
"""Contrastive (SimCLR-style) loss on 8 Trainium2 NeuronCores.

Math (matches the reference exactly):
  P = concat(projection1, projection2)            # [8192, 256]
  sim = cos_sim(P_i, P_j); diag masked to -1e9; logits = sim / 0.5
  labels = arange(2B)  -> picks the masked diagonal, so
  loss = -mean_i( logp_ii ),  logp_ii = f32(-2e9 - lse_i),
  lse_i = log(sum_{j != i} exp(2*sim_ij))

Distribution: data-parallel over the 8192 rows.  Each core receives the
full projection matrix (row-major fp32 for norms + pre-transposed bf16
for the matmul operand) plus its own 1024-row block.  On chip it:
  - computes row norms (DVE square+reduce, Newton rsqrt -- no ScalarE),
  - scales the transposed operand by 1/norm (bf16),
  - matmuls its row block against all 8192 columns (bf16, fp32 PSUM),
  - streams exp through ScalarE with fused row-sum accumulation,
  - subtracts the diagonal term and takes log.
Host all-reduces the per-row lse partials and applies the reference's
fp32 arithmetic for the final mean.
"""

import sys

for _p in ("/opt/trn_rl_repo", "/root/.axon_site/_ro/trn_rl_repo"):
    if _p not in sys.path:
        sys.path.append(_p)

import numpy as np

import concourse.bacc as bacc
import concourse.tile as tile
from concourse import mybir
from concourse import bass_utils

F32 = mybir.dt.float32
BF16 = mybir.dt.bfloat16
I32 = mybir.dt.int32
AF = mybir.ActivationFunctionType
ALU = mybir.AluOpType

N_CORES = 8
B = 8192          # total rows (2 * batch)
D = 256           # projection dim
BLK = B // N_CORES        # 1024 rows per core
M_TILES = BLK // 128      # 8 row tiles per core
N_COLS = 512              # matmul free dim (one PSUM bank)
GROUP = 2048              # ACT exp batch (4 PSUM banks) = one column group
N_GROUPS = B // GROUP     # 4
N_PER_GROUP = GROUP // N_COLS  # 4
U = 16                    # consecutive rows per partition in stats loads
RSQRT_MAGIC = 0x5F3759DF


def _newton_rsqrt(nc, pool, out_rn, s):
    """out_rn = 1/sqrt(s), entirely on VectorE (fp32).

    Quake-style bit seed + 2 Newton iterations (~5e-6 rel err).  Keeps
    ScalarE free for exp and avoids sqrt<->exp table reloads.
    """
    p, w = s.shape
    ibits = pool.tile([p, w], I32, name="ibits", tag="rsq_i", bufs=2)
    nc.vector.tensor_scalar(
        out=ibits, in0=s.bitcast(I32), scalar1=1, scalar2=None,
        op0=ALU.arith_shift_right,
    )
    nc.vector.tensor_scalar(
        out=ibits, in0=ibits, scalar1=-1, scalar2=RSQRT_MAGIC,
        op0=ALU.mult, op1=ALU.add,
    )
    y = ibits.bitcast(F32)
    t1 = pool.tile([p, w], F32, name="t1", tag="rsq_t1", bufs=2)
    for _ in range(2):
        nc.vector.tensor_mul(t1, y, y)
        nc.vector.tensor_mul(t1, t1, s)
        nc.vector.tensor_scalar(
            out=t1, in0=t1, scalar1=-0.5, scalar2=1.5,
            op0=ALU.mult, op1=ALU.add,
        )
        nc.vector.tensor_mul(y, y, t1)
    nc.vector.tensor_copy(out_rn, y)


def _emit(tc, p_stats, pt, p_blk, eye_in, lse_out):
    nc = tc.nc

    persist = tc.alloc_tile_pool(name="persist", bufs=1)
    pin = tc.alloc_tile_pool(name="pin", bufs=2)
    work = tc.alloc_tile_pool(name="work", bufs=2)
    dram = tc.alloc_tile_pool(name="dram", bufs=1, space="DRAM")
    epool = tc.alloc_tile_pool(name="epool", bufs=2)

    # Persistent tensors
    qt0 = persist.tile([128, B], BF16, tag="qt0", name="qt0")
    qt1 = persist.tile([128, B], BF16, tag="qt1", name="qt1")
    bt0 = persist.tile([128, BLK], BF16, tag="bt0", name="bt0")
    bt1 = persist.tile([128, BLK], BF16, tag="bt1", name="bt1")
    q_b = persist.tile([128, M_TILES, D], BF16, tag="q_b", name="q_b")
    rn_f = persist.tile([128, 64], F32, tag="rn_f", name="rn_f")
    rn_b = persist.tile([128, M_TILES], F32, tag="rn_b", name="rn_b")
    selfdot = persist.tile([128, M_TILES], F32, tag="selfdot", name="selfdot")
    sums = persist.tile([128, N_GROUPS * M_TILES], F32, tag="sums", name="sums")
    rowsum = persist.tile([128, M_TILES], F32, tag="rowsum", name="rowsum")
    exps = persist.tile([128, M_TILES], F32, tag="exps", name="exps")
    lse = persist.tile([128, M_TILES], F32, tag="lse", name="lse")
    dram_rn = dram.tile([B], F32, tag="dram_rn", name="dram_rn")

    # ---- This core's row block: norms, scale, self-dot, transpose ----
    pb = p_blk.rearrange("(t p) d -> t p d", p=128)    # [8, 128, 256]
    blk = persist.tile([128, M_TILES, D], F32, tag="blk", name="blk")
    eye = persist.tile([128, 128], BF16, tag="eye", name="eye")
    nc.gpsimd.dma_start(out=eye, in_=eye_in)
    for t in range(M_TILES):
        nc.gpsimd.dma_start(out=blk[:, t, :], in_=pb[t])
    sq_b = work.tile([128, M_TILES, D], BF16, name="sq_b", tag="sq_b", bufs=1)
    nc.vector.tensor_mul(sq_b, blk, blk)
    stats_b = work.tile([128, M_TILES], F32, name="stats_b", tag="stats_b", bufs=1)
    nc.vector.tensor_reduce(stats_b, sq_b, axis=mybir.AxisListType.X, op=ALU.add)
    _newton_rsqrt(nc, work, rn_b, stats_b)
    for t in range(M_TILES):
        nc.vector.tensor_scalar_mul(q_b[:, t, :], blk[:, t, :], rn_b[:, t : t + 1])
    sq_b2 = work.tile([128, M_TILES, D], BF16, name="sq_b2", tag="sq_b", bufs=1)
    nc.vector.tensor_mul(sq_b2, q_b, q_b)
    nc.vector.tensor_reduce(selfdot, sq_b2, axis=mybir.AxisListType.X, op=ALU.add)
    # Transpose the block on the (otherwise idle) tensor engine; copy the
    # PSUM results to SBUF on the scalar engine.  This keeps the slow DMA
    # xbar out of the picture and frees the main loop from DMA-queue deps.
    tp_psum = tc.alloc_tile_pool(name="tp_psum", bufs=4, space="PSUM")
    for t in range(M_TILES):
        for half, btk in ((0, bt0), (1, bt1)):
            tp = tp_psum.tile([128, 128], BF16, name="tp")
            nc.tensor.transpose(tp, q_b[:, t, half * 128 : half * 128 + 128], eye)
            nc.scalar.copy(out=btk[:, t * 128 : (t + 1) * 128], in_=tp)
    tp_psum.release()
    psum_pool = tc.alloc_tile_pool(name="psum", bufs=2, space="PSUM")

    # ---- Full-matrix norms + scaled transposed operand, one group at a
    # time (group g covers columns [2048g, 2048(g+1)) = rows with the
    # same indices; the u=16 interleave keeps j-order identity) ----
    # stats load: row j = 2048t + 16p + u  ->  tile t, partition p, slot u
    ps4 = p_stats.rearrange("(t p u) d -> t p (u d)", p=128, u=U)  # [4,128,4096]
    # rn store: dram_rn[2048t + 16p + u] <- rn_small[p, 16t + u]
    rn_store = dram_rn.rearrange("(t p u) -> t p u", p=128, u=U)   # [4,128,16]

    def normalize_group(g):
        pst = pin.tile([128, U * D], F32, name="pst", tag="pst", bufs=2)
        nc.sync.dma_start(out=pst, in_=ps4[g])
        sq = work.tile([128, U * D], BF16, name="sq", tag="sq", bufs=2)
        nc.vector.tensor_mul(sq, pst, pst)
        nc.vector.tensor_reduce(
            rn_f[:, g * U : (g + 1) * U],
            sq.rearrange("p (u d) -> p u d", u=U),
            axis=mybir.AxisListType.X,
            op=ALU.add,
        )
        _newton_rsqrt(
            nc, work, rn_f[:, g * U : (g + 1) * U], rn_f[:, g * U : (g + 1) * U]
        )
        nc.sync.dma_start(
            out=rn_store[g],
            in_=rn_f[:, g * U : (g + 1) * U].rearrange("p (t u) -> p t u", u=U),
        )
        rnb = work.tile([128, GROUP], F32, name="rnb", tag="rnb", bufs=2)
        nc.sync.dma_start(
            out=rnb,
            in_=dram_rn[g * GROUP : (g + 1) * GROUP].partition_broadcast(128),
        )
        for k, qtk in enumerate((qt0, qt1)):
            ptc = pin.tile([128, GROUP], F32, name="ptc", tag="ptc", bufs=4)
            nc.gpsimd.dma_start(
                out=ptc,
                in_=pt[k * 128 : (k + 1) * 128, g * GROUP : (g + 1) * GROUP],
            )
            nc.vector.tensor_mul(
                qtk[:, g * GROUP : (g + 1) * GROUP], ptc, rnb
            )

    normalize_group(0)

    # ---- Main loop: S-block matmuls + fused exp/row-sum ----
    for g in range(N_GROUPS):
        if g + 1 < N_GROUPS:
            normalize_group(g + 1)
        for m in range(M_TILES):
            ps = psum_pool.tile([128, GROUP], F32, name="ps")
            for n4 in range(N_PER_GROUP):
                col = g * GROUP + n4 * N_COLS
                for k, (btk, qtk) in enumerate(((bt0, qt0), (bt1, qt1))):
                    nc.tensor.matmul(
                        ps[:, n4 * N_COLS : (n4 + 1) * N_COLS],
                        btk[:, m * 128 : (m + 1) * 128],
                        qtk[:, col : col + N_COLS],
                        start=(k == 0),
                        stop=(k == 1),
                    )
            esc = epool.tile([128, GROUP], BF16, name="esc")
            nc.scalar.activation(
                out=esc,
                in_=ps,
                func=AF.Exp,
                scale=2.0,
                accum_out=sums[:, g * M_TILES + m : g * M_TILES + m + 1],
            )

    # ---- Epilogue: rowsum over groups, drop diagonal, log, write out ----
    sums3 = sums.rearrange("p (g m) -> p m g", g=N_GROUPS)
    nc.vector.tensor_reduce(rowsum, sums3, axis=mybir.AxisListType.X, op=ALU.add)
    nc.scalar.activation(out=exps, in_=selfdot, func=AF.Exp, scale=2.0)
    nc.vector.tensor_tensor(lse, rowsum, exps, op=ALU.subtract)
    nc.scalar.activation(out=lse, in_=lse, func=AF.Ln)
    nc.sync.dma_start(out=lse_out, in_=lse)

    for p in (epool, psum_pool, dram, work, pin, persist):
        p.release()


_BUILT = None


def _build():
    global _BUILT
    if _BUILT is None:
        nc = bacc.Bacc("TRN2", target_bir_lowering=False, debug=False,
                       num_devices=N_CORES)
        p_stats = nc.dram_tensor("p_stats", [B, D], F32, kind="ExternalInput").ap()
        pt = nc.dram_tensor("pt", [D, B], F32, kind="ExternalInput").ap()
        eye = nc.dram_tensor("eye", [128, 128], BF16, kind="ExternalInput").ap()
        p_blk = nc.dram_tensor("p_blk", [BLK, D], F32, kind="ExternalInput").ap()
        lse_out = nc.dram_tensor("lse_out", [128, M_TILES], F32,
                                 kind="ExternalOutput").ap()
        with tile.TileContext(nc) as tc:
            _emit(tc, p_stats, pt, p_blk, eye, lse_out)
        nc.finalize()
        _BUILT = nc
    return _BUILT


def run_on_hw(P, **spmd_kwargs):
    import jax.numpy as jnp

    nc = _build()
    pt_f32 = np.ascontiguousarray(P.T)
    eye = np.asarray(jnp.eye(128, dtype=jnp.bfloat16))
    in_maps = [
        {
            "p_stats": P,
            "pt": pt_f32,
            "p_blk": np.ascontiguousarray(P[c * BLK : (c + 1) * BLK]),
            "eye": eye,
        }
        for c in range(N_CORES)
    ]
    return bass_utils.run_bass_kernel_spmd(
        nc, in_maps, core_ids=list(range(N_CORES)), **spmd_kwargs
    )


def kernel(embedding1, embedding2, projection1, projection2):
    import jax.numpy as jnp

    # embeddings are unused by the reference computation
    P = np.ascontiguousarray(
        np.concatenate([projection1, projection2], axis=0), dtype=np.float32
    )
    res = run_on_hw(P)
    # reassemble per-row lse: core c, tile column m, partition p ->
    # global row c*1024 + m*128 + p
    lse_rows = np.empty(B, np.float32)
    for c in range(N_CORES):
        arr = np.asarray(res.results[c]["lse_out"])  # [128, M_TILES]
        lse_rows[c * BLK : (c + 1) * BLK] = arr.T.reshape(-1)
    # Reference fp32 semantics: logp_ii = f32(-2e9 - lse_i) (== -2e9 for
    # any |lse| < 128), then loss = -mean(logp) with the platform's XLA
    # fp32 reduction -- reproduce it bit-for-bit.
    logp = (np.float32(-2.0e9) - lse_rows).astype(np.float32)
    loss = -jnp.mean(jnp.asarray(logp))
    return np.asarray(loss)



# revision 3
# speedup vs baseline: 1.1868x; 1.1868x over previous
"""Contrastive (SimCLR-style) loss on 8 Trainium2 NeuronCores.

Math (matches the reference exactly):
  P = concat(projection1, projection2)            # [8192, 256]
  sim = cos_sim(P_i, P_j); diag masked to -1e9; logits = sim / 0.5
  labels = arange(2B)  -> picks the masked diagonal, so
  loss = -mean_i( logp_ii ),  logp_ii = f32(-2e9 - lse_i),
  lse_i = log(sum_{j != i} exp(2*sim_ij))

Distribution: data-parallel over the 8192 rows; every core holds the full
bf16 P^T as the moving matmul operand and its own 1024-row slice as the
stationary operand.  Per core:
  - global column norms from a row-major bf16 copy (DVE square + free-dim
    reduce in u=16 interleaved layout, Newton rsqrt),
  - Q^T = P^T * (1/n_j) via a DRAM-bounced partition broadcast (bf16, DVE),
  - own-row block normalized the same way (stationary = Q^T column slice,
    so there is no on-chip transpose at all),
  - matmul row-block x all 8192 columns (bf16, fp32 PSUM, 2048-col groups),
  - exp streamed through ScalarE with fused row-sum accumulation,
  - rowsum - e^2 (the diagonal term of a normalized Gram matrix is exactly
    cos_sim = 1 -> exp(2) ), then log.
Host applies the reference's fp32 arithmetic for the final mean.
"""

import sys

for _p in ("/opt/trn_rl_repo", "/root/.axon_site/_ro/trn_rl_repo"):
    if _p not in sys.path:
        sys.path.append(_p)

import numpy as np

import concourse.bacc as bacc
import concourse.tile as tile
from concourse import mybir
from concourse import bass_utils

F32 = mybir.dt.float32
BF16 = mybir.dt.bfloat16
I32 = mybir.dt.int32
AF = mybir.ActivationFunctionType
ALU = mybir.AluOpType

N_CORES = 8
B = 8192          # total rows (2 * batch)
D = 256           # projection dim
BLK = B // N_CORES        # 1024 rows per core
M_TILES = BLK // 128      # 8 row tiles per core
N_COLS = 512              # matmul free dim (one PSUM bank)
GROUP = 2048              # ACT exp batch (4 PSUM banks) = one column group
N_GROUPS = B // GROUP     # 4
N_PER_GROUP = GROUP // N_COLS  # 4
U = 16                    # consecutive rows per partition in stats loads
UO = BLK // 128           # 8: rows per partition in own-block stats load
RSQRT_MAGIC = 0x5F3759DF
E2 = 7.38905609893065     # exp(2): the masked diagonal's exp term


def _newton_rsqrt(nc, pool, out_rn, s):
    """out_rn = 1/sqrt(s), entirely on VectorE (fp32).

    Quake-style bit seed + 2 Newton iterations (~5e-6 rel err).  Keeps
    ScalarE free for exp and avoids sqrt<->exp table reloads.
    """
    p, w = s.shape
    ibits = pool.tile([p, w], I32, name="ibits", tag="rsq_i", bufs=2)
    nc.vector.tensor_scalar(
        out=ibits, in0=s.bitcast(I32), scalar1=1, scalar2=None,
        op0=ALU.arith_shift_right,
    )
    nc.vector.tensor_scalar(
        out=ibits, in0=ibits, scalar1=-1, scalar2=RSQRT_MAGIC,
        op0=ALU.mult, op1=ALU.add,
    )
    y = ibits.bitcast(F32)
    t1 = pool.tile([p, w], F32, name="t1", tag="rsq_t1", bufs=2)
    for _ in range(2):
        nc.vector.tensor_mul(t1, y, y)
        nc.vector.tensor_mul(t1, t1, s)
        nc.vector.tensor_scalar(
            out=t1, in0=t1, scalar1=-0.5, scalar2=1.5,
            op0=ALU.mult, op1=ALU.add,
        )
        nc.vector.tensor_mul(y, y, t1)
    nc.vector.tensor_copy(out_rn, y)


def _emit(tc, pt, pr, pblkt, pblk, lse_out):
    nc = tc.nc

    persist = tc.alloc_tile_pool(name="persist", bufs=1)
    pin = tc.alloc_tile_pool(name="pin", bufs=2)
    work = tc.alloc_tile_pool(name="work", bufs=2)
    dram = tc.alloc_tile_pool(name="dram", bufs=1, space="DRAM")
    epool = tc.alloc_tile_pool(name="epool", bufs=2)
    psum_pool = tc.alloc_tile_pool(name="psum", bufs=2, space="PSUM")

    # Persistent tensors
    ptk = [persist.tile([128, B], BF16, tag=f"pt{k}", name=f"pt{k}")
           for k in range(2)]
    qtk = [persist.tile([128, B], BF16, tag=f"qt{k}", name=f"qt{k}")
           for k in range(2)]
    qsk = [persist.tile([128, BLK], BF16, tag=f"qs{k}", name=f"qs{k}")
           for k in range(2)]
    rn_f = persist.tile([128, 64], F32, tag="rn_f", name="rn_f")
    rn_b16 = persist.tile([128, 64], BF16, tag="rn_b16", name="rn_b16")
    nblk = persist.tile([128, UO], F32, tag="nblk", name="nblk")
    rno_b = persist.tile([128, UO], BF16, tag="rno_b", name="rno_b")
    rnob = persist.tile([128, BLK], BF16, tag="rnob", name="rnob")
    sums = persist.tile([128, N_GROUPS * M_TILES], F32, tag="sums", name="sums")
    rowsum = persist.tile([128, M_TILES], F32, tag="rowsum", name="rowsum")
    lse = persist.tile([128, M_TILES], F32, tag="lse", name="lse")
    dram_rn = dram.tile([B], BF16, tag="dram_rn", name="dram_rn")
    dram_rno = dram.tile([BLK], BF16, tag="dram_rno", name="dram_rno")

    # ---- Bulk loads: full bf16 P^T (moving operand) + own-block slice ----
    for k in range(2):
        for g in range(N_GROUPS):
            nc.gpsimd.dma_start(
                out=ptk[k][:, g * GROUP : (g + 1) * GROUP],
                in_=pt[k * 128 : (k + 1) * 128, g * GROUP : (g + 1) * GROUP],
            )
    pblkt_t = [persist.tile([128, BLK], BF16, tag=f"pbt{k}", name=f"pbt{k}")
               for k in range(2)]
    for k in range(2):
        nc.gpsimd.dma_start(out=pblkt_t[k], in_=pblkt[k * 128 : (k + 1) * 128, :])

    # ---- Own-block norms: row i = 128*p + u lives at partition p, slot u
    # after the (p u) d -> p (u d) interleave; compact layout keeps Newton
    # cheap and the DRAM store contiguous. ----
    pbo = pblk.rearrange("(p u) d -> p (u d)", p=128, u=UO)
    pblk_il = pin.tile([128, UO * D], BF16, name="pblk_il", tag="pblk_il", bufs=1)
    nc.sync.dma_start(out=pblk_il, in_=pbo)
    sq_o = work.tile([128, UO * D], BF16, name="sq_o", tag="sq_o", bufs=1)
    nc.vector.tensor_mul(sq_o, pblk_il, pblk_il)
    nc.vector.tensor_reduce(
        nblk, sq_o.rearrange("p (u d) -> p u d", u=UO),
        axis=mybir.AxisListType.X, op=ALU.add,
    )
    _newton_rsqrt(nc, work, nblk, nblk)
    nc.vector.tensor_copy(rno_b, nblk)
    nc.sync.dma_start(
        out=dram_rno.rearrange("(p u) -> p u", p=128), in_=rno_b
    )
    nc.sync.dma_start(out=rnob, in_=dram_rno.partition_broadcast(128))
    for k in range(2):
        nc.vector.tensor_mul(qsk[k], pblkt_t[k], rnob)

    # ---- Global norms + scaled moving operand, one 2048-column group at a
    # time (row j = 2048g + 16p + u -> group g, partition p, slot u) ----
    ps4 = pr.rearrange("(g p u) d -> g p (u d)", p=128, u=U)   # [4,128,4096]
    rn_store = dram_rn.rearrange("(g p u) -> g p u", p=128, u=U)

    def normalize_group(g):
        pst = pin.tile([128, U * D], BF16, name="pst", tag="pst", bufs=2)
        nc.sync.dma_start(out=pst, in_=ps4[g])
        sq = work.tile([128, U * D], BF16, name="sq", tag="sq", bufs=2)
        nc.vector.tensor_mul(sq, pst, pst)
        nc.vector.tensor_reduce(
            rn_f[:, g * U : (g + 1) * U],
            sq.rearrange("p (u d) -> p u d", u=U),
            axis=mybir.AxisListType.X,
            op=ALU.add,
        )
        _newton_rsqrt(
            nc, work, rn_f[:, g * U : (g + 1) * U], rn_f[:, g * U : (g + 1) * U]
        )
        nc.vector.tensor_copy(
            rn_b16[:, g * U : (g + 1) * U], rn_f[:, g * U : (g + 1) * U]
        )
        nc.sync.dma_start(
            out=rn_store[g],
            in_=rn_b16[:, g * U : (g + 1) * U],
        )
        rnb = work.tile([128, GROUP], BF16, name="rnb", tag="rnb", bufs=2)
        nc.sync.dma_start(
            out=rnb,
            in_=dram_rn[g * GROUP : (g + 1) * GROUP].partition_broadcast(128),
        )
        for k in range(2):
            nc.vector.tensor_mul(
                qtk[k][:, g * GROUP : (g + 1) * GROUP],
                ptk[k][:, g * GROUP : (g + 1) * GROUP],
                rnb,
            )

    normalize_group(0)

    # ---- Main loop: S-block matmuls + fused exp/row-sum ----
    for g in range(N_GROUPS):
        if g + 1 < N_GROUPS:
            normalize_group(g + 1)
        for m in range(M_TILES):
            ps = psum_pool.tile([128, GROUP], F32, name="ps")
            for n4 in range(N_PER_GROUP):
                col = g * GROUP + n4 * N_COLS
                for k in range(2):
                    nc.tensor.matmul(
                        ps[:, n4 * N_COLS : (n4 + 1) * N_COLS],
                        qsk[k][:, m * 128 : (m + 1) * 128],
                        qtk[k][:, col : col + N_COLS],
                        start=(k == 0),
                        stop=(k == 1),
                    )
            esc = epool.tile([128, GROUP], BF16, name="esc")
            nc.scalar.activation(
                out=esc,
                in_=ps,
                func=AF.Exp,
                scale=2.0,
                accum_out=sums[:, g * M_TILES + m : g * M_TILES + m + 1],
            )

    # ---- Epilogue: rowsum over groups, drop diagonal, log, write out ----
    sums3 = sums.rearrange("p (g m) -> p m g", g=N_GROUPS)
    nc.vector.tensor_reduce(rowsum, sums3, axis=mybir.AxisListType.X, op=ALU.add)
    nc.vector.tensor_scalar(
        out=lse, in0=rowsum, scalar1=E2, scalar2=None, op0=ALU.subtract,
    )
    nc.scalar.activation(out=lse, in_=lse, func=AF.Ln)
    nc.sync.dma_start(out=lse_out, in_=lse)

    for p in (psum_pool, epool, dram, work, pin, persist):
        p.release()


_BUILT = None


def _build():
    global _BUILT
    if _BUILT is None:
        nc = bacc.Bacc("TRN2", target_bir_lowering=False, debug=False,
                       num_devices=N_CORES)
        pt = nc.dram_tensor("pt", [D, B], BF16, kind="ExternalInput").ap()
        pr = nc.dram_tensor("pr", [B, D], BF16, kind="ExternalInput").ap()
        pblkt = nc.dram_tensor("pblkt", [D, BLK], BF16, kind="ExternalInput").ap()
        pblk = nc.dram_tensor("pblk", [BLK, D], BF16, kind="ExternalInput").ap()
        lse_out = nc.dram_tensor("lse_out", [128, M_TILES], F32,
                                 kind="ExternalOutput").ap()
        with tile.TileContext(nc) as tc:
            _emit(tc, pt, pr, pblkt, pblk, lse_out)
        nc.finalize()
        _BUILT = nc
    return _BUILT


def run_on_hw(P, **spmd_kwargs):
    import jax.numpy as jnp

    nc = _build()
    P_b = np.asarray(jnp.asarray(P, dtype=jnp.bfloat16))
    PT_b = np.ascontiguousarray(P_b.T)
    in_maps = [
        {
            "pt": PT_b,
            "pr": P_b,
            "pblkt": np.ascontiguousarray(PT_b[:, c * BLK : (c + 1) * BLK]),
            "pblk": np.ascontiguousarray(P_b[c * BLK : (c + 1) * BLK]),
        }
        for c in range(N_CORES)
    ]
    return bass_utils.run_bass_kernel_spmd(
        nc, in_maps, core_ids=list(range(N_CORES)), **spmd_kwargs
    )


def kernel(embedding1, embedding2, projection1, projection2):
    import jax.numpy as jnp

    # embeddings are unused by the reference computation
    P = np.ascontiguousarray(
        np.concatenate([projection1, projection2], axis=0), dtype=np.float32
    )
    res = run_on_hw(P)
    # reassemble per-row lse: core c, tile column m, partition p ->
    # global row c*1024 + m*128 + p
    lse_rows = np.empty(B, np.float32)
    for c in range(N_CORES):
        arr = np.asarray(res.results[c]["lse_out"])  # [128, M_TILES]
        lse_rows[c * BLK : (c + 1) * BLK] = arr.T.reshape(-1)
    # Reference fp32 semantics: logp_ii = f32(-2e9 - lse_i) (== -2e9 for
    # any |lse| < 128), then loss = -mean(logp) with the platform's XLA
    # fp32 reduction -- reproduce it bit-for-bit.
    logp = (np.float32(-2.0e9) - lse_rows).astype(np.float32)
    loss = -jnp.mean(jnp.asarray(logp))
    return np.asarray(loss)


# revision 9
# speedup vs baseline: 1.2124x; 1.0216x over previous
"""Contrastive (SimCLR-style) loss on 8 Trainium2 NeuronCores.

Math (matches the reference exactly):
  P = concat(projection1, projection2)            # [8192, 256]
  sim = cos_sim(P_i, P_j); diag masked to -1e9; logits = sim / 0.5
  labels = arange(2B)  -> picks the masked diagonal, so
  loss = -mean_i( logp_ii ),  logp_ii = f32(-2e9 - lse_i),
  lse_i = log(sum_{j != i} exp(2*sim_ij))

Distribution: data-parallel over the 8192 rows; every core holds the full
bf16 P^T as the moving matmul operand and its own RAW 1024-column slice as
the stationary operand (no on-chip transpose).  Per core:
  - column norms from a row-major bf16 copy (DVE square + free-dim reduce
    in u=16 interleaved layout, bf16 partials, Newton rsqrt per group),
  - Q^T = P^T * (1/n_j) via a DRAM-bounced partition broadcast (bf16, DVE),
  - the row-side 1/n_i never touches the operands: it rides the ScalarE
    activation's per-partition scale (exp(scale_i * psum), scale_i = 2/n_i),
  - matmul row-block x all 8192 columns (bf16, fp32 PSUM, 2048-col groups),
  - exp streamed through ScalarE with fused row-sum accumulation,
  - rowsum - e^2 (diagonal of the normalized Gram is cos=1 -> exp(2)), log.
Groups 1-3's norm prep is held back with tile_wait_until so the greedy
tile scheduler cannot wedge big DVE ops into the group-0 critical chain.
Host applies the reference's fp32 arithmetic for the final mean.
"""

import sys

for _p in ("/opt/trn_rl_repo", "/root/.axon_site/_ro/trn_rl_repo"):
    if _p not in sys.path:
        sys.path.append(_p)

import numpy as np

import concourse.bacc as bacc
import concourse.tile as tile
from concourse import mybir
from concourse import bass_utils

F32 = mybir.dt.float32
BF16 = mybir.dt.bfloat16
I32 = mybir.dt.int32
AF = mybir.ActivationFunctionType
ALU = mybir.AluOpType

N_CORES = 8
B = 8192          # total rows (2 * batch)
D = 256           # projection dim
BLK = B // N_CORES        # 1024 rows per core
M_TILES = BLK // 128      # 8 row tiles per core
N_COLS = 512              # matmul free dim (one PSUM bank)
GROUP = 2048              # ACT exp batch (4 PSUM banks) = one column group
N_GROUPS = B // GROUP     # 4
N_PER_GROUP = GROUP // N_COLS  # 4
U = 16                    # consecutive rows per partition in stats loads
UO = BLK // 128           # 8: row tiles in the own-block stats load
RSQRT_MAGIC = 0x5F3759DF
E2 = 7.38905609893065     # exp(2): the masked diagonal's exp term


def _newton_rsqrt(nc, pool, out_rn, s, tag):
    """out_rn = 1/sqrt(s), entirely on VectorE (fp32).

    Quake-style bit seed + 2 Newton iterations (~5e-6 rel err).  Keeps
    ScalarE free for exp and avoids sqrt<->exp table reloads.
    """
    p, w = s.shape
    ibits = pool.tile([p, w], I32, name="ibits", tag=f"rsqi_{tag}", bufs=1)
    nc.vector.tensor_scalar(
        out=ibits, in0=s.bitcast(I32), scalar1=1, scalar2=None,
        op0=ALU.arith_shift_right,
    )
    nc.vector.tensor_scalar(
        out=ibits, in0=ibits, scalar1=-1, scalar2=RSQRT_MAGIC,
        op0=ALU.mult, op1=ALU.add,
    )
    y = ibits.bitcast(F32)
    t1 = pool.tile([p, w], F32, name="t1", tag=f"rsqt_{tag}", bufs=1)
    for _ in range(2):
        nc.vector.tensor_mul(t1, y, y)
        nc.vector.tensor_mul(t1, t1, s)
        nc.vector.tensor_scalar(
            out=t1, in0=t1, scalar1=-0.5, scalar2=1.5,
            op0=ALU.mult, op1=ALU.add,
        )
        nc.vector.tensor_mul(y, y, t1)
    nc.vector.tensor_copy(out_rn, y)


def _emit(tc, pt, pr, pblkt, pblk, lse_out):
    nc = tc.nc

    persist = tc.alloc_tile_pool(name="persist", bufs=1)
    pin = tc.alloc_tile_pool(name="pin", bufs=2)
    work = tc.alloc_tile_pool(name="work", bufs=2)
    dram = tc.alloc_tile_pool(name="dram", bufs=1, space="DRAM")
    epool = tc.alloc_tile_pool(name="epool", bufs=2)
    psum_pool = tc.alloc_tile_pool(name="psum", bufs=2, space="PSUM")

    # Persistent tensors
    ptk = [persist.tile([128, B], BF16, tag=f"pt{k}", name=f"pt{k}")
           for k in range(2)]
    qtk = [persist.tile([128, B], BF16, tag=f"qt{k}", name=f"qt{k}")
           for k in range(2)]
    pblkt_t = [persist.tile([128, BLK], BF16, tag=f"pbt{k}", name=f"pbt{k}")
               for k in range(2)]
    rn_f = persist.tile([128, 64], F32, tag="rn_f", name="rn_f")
    rn_b16 = persist.tile([128, 64], BF16, tag="rn_b16", name="rn_b16")
    nblk = persist.tile([128, UO], BF16, tag="nblk", name="nblk")
    nblk_f = persist.tile([128, UO], F32, tag="nblk_f", name="nblk_f")
    scv = persist.tile([128, UO], F32, tag="scv", name="scv")
    sums = persist.tile([128, N_GROUPS * M_TILES], F32, tag="sums", name="sums")
    rowsum = persist.tile([128, M_TILES], F32, tag="rowsum", name="rowsum")
    lse = persist.tile([128, M_TILES], F32, tag="lse", name="lse")
    dram_rn = dram.tile([B], BF16, tag="dram_rn", name="dram_rn")

    # ---- Bulk loads: full bf16 P^T (moving operand) + own-block slice ----
    for k in range(2):
        for g in range(N_GROUPS):
            nc.gpsimd.dma_start(
                out=ptk[k][:, g * GROUP : (g + 1) * GROUP],
                in_=pt[k * 128 : (k + 1) * 128, g * GROUP : (g + 1) * GROUP],
            )
    for k in range(2):
        nc.gpsimd.dma_start(out=pblkt_t[k], in_=pblkt[k * 128 : (k + 1) * 128, :])

    # ---- Global column norms, one 2048-column group at a time.
    # Row-major interleave: row j = 2048g + 16p + u -> partition p, slot u.
    # Squares + reduce on DVE in bf16 (2x mode), Newton per group. ----
    ps4 = pr.rearrange("(g p u) d -> g p (u d)", p=128, u=U)   # [4,128,4096]
    rn_store = dram_rn.rearrange("(g p u) -> g p u", p=128, u=U)

    def normalize_group(g):
        gsl = slice(g * U, (g + 1) * U)
        pst = pin.tile([128, U * D], BF16, name="pst", tag="pst", bufs=2)
        nc.sync.dma_start(out=pst, in_=ps4[g])
        sq = work.tile([128, U * D], BF16, name="sq", tag="sq", bufs=2)
        nc.vector.tensor_mul(sq, pst, pst)
        nsq_b = work.tile([128, U], BF16, name="nsq_b", tag="nsq_b", bufs=2)
        with nc.allow_low_precision(reason="bf16 norm^2 partials, 0.4% ok"):
            nc.vector.tensor_reduce(
                nsq_b, sq.rearrange("p (u d) -> p u d", u=U),
                axis=mybir.AxisListType.X, op=ALU.add,
            )
        nc.vector.tensor_copy(rn_f[:, gsl], nsq_b)
        _newton_rsqrt(nc, work, rn_f[:, gsl], rn_f[:, gsl], tag=f"g{g}")
        nc.vector.tensor_copy(rn_b16[:, gsl], rn_f[:, gsl])
        nc.sync.dma_start(out=rn_store[g], in_=rn_b16[:, gsl])
        rnb = work.tile([128, GROUP], BF16, name="rnb", tag="rnb", bufs=2)
        nc.sync.dma_start(
            out=rnb,
            in_=dram_rn[g * GROUP : (g + 1) * GROUP].partition_broadcast(128),
        )
        for k in range(2):
            nc.vector.tensor_mul(
                qtk[k][:, g * GROUP : (g + 1) * GROUP],
                ptk[k][:, g * GROUP : (g + 1) * GROUP],
                rnb,
            )

    normalize_group(0)

    # ---- Own-block row norms -> per-partition ACT scale 2/n_i.
    # Row i = 128u + p -> partition p, slot u, so scv[:, m] lines up with
    # the psum partition dim of row-tile m.  Off the critical path: first
    # needed by the first EXP, not the first matmul. ----
    pbo = pblk.rearrange("(u p) d -> p u d", p=128, u=UO)
    pblk_il = pin.tile([128, UO * D], BF16, name="pblk_il", tag="pblk_il", bufs=1)
    nc.sync.dma_start(
        out=pblk_il.rearrange("p (u d) -> p u d", u=UO), in_=pbo
    )
    sq_o = work.tile([128, UO * D], BF16, name="sq_o", tag="sq_o", bufs=1)
    nc.vector.tensor_mul(sq_o, pblk_il, pblk_il)
    with nc.allow_low_precision(reason="bf16 norm^2 partials, 0.4% ok"):
        nc.vector.tensor_reduce(
            nblk, sq_o.rearrange("p (u d) -> p u d", u=UO),
            axis=mybir.AxisListType.X, op=ALU.add,
        )
    nc.vector.tensor_copy(nblk_f, nblk)
    _newton_rsqrt(nc, work, nblk_f, nblk_f, tag="own")
    nc.vector.tensor_scalar(
        out=scv, in0=nblk_f, scalar1=2.0, scalar2=None, op0=ALU.mult,
    )

    # ---- Main loop: S-block matmuls + fused exp/row-sum.  Group g>=1 norm
    # prep is released ~when group g-1's exps start so the greedy scheduler
    # can't stuff it into the group-0 critical chain. ----
    for g in range(N_GROUPS):
        if g + 1 < N_GROUPS:
            with tc.tile_wait_until(0.008 + 0.014 * g):
                normalize_group(g + 1)
        for m in range(M_TILES):
            ps = psum_pool.tile([128, GROUP], F32, name="ps")
            for n4 in range(N_PER_GROUP):
                col = g * GROUP + n4 * N_COLS
                for k in range(2):
                    nc.tensor.matmul(
                        ps[:, n4 * N_COLS : (n4 + 1) * N_COLS],
                        pblkt_t[k][:, m * 128 : (m + 1) * 128],
                        qtk[k][:, col : col + N_COLS],
                        start=(k == 0),
                        stop=(k == 1),
                    )
            esc = epool.tile([128, GROUP], BF16, name="esc")
            nc.scalar.activation(
                out=esc,
                in_=ps,
                func=AF.Exp,
                scale=scv[:, m : m + 1],
                accum_out=sums[:, g * M_TILES + m : g * M_TILES + m + 1],
            )

    # ---- Epilogue: rowsum over groups, drop diagonal, log, write out ----
    sums3 = sums.rearrange("p (g m) -> p m g", g=N_GROUPS)
    nc.vector.tensor_reduce(rowsum, sums3, axis=mybir.AxisListType.X, op=ALU.add)
    nc.vector.tensor_scalar(
        out=lse, in0=rowsum, scalar1=E2, scalar2=None, op0=ALU.subtract,
    )
    nc.scalar.activation(out=lse, in_=lse, func=AF.Ln)
    nc.sync.dma_start(out=lse_out, in_=lse)

    for p in (psum_pool, epool, dram, work, pin, persist):
        p.release()


_BUILT = None


def _build():
    global _BUILT
    if _BUILT is None:
        nc = bacc.Bacc("TRN2", target_bir_lowering=False, debug=False,
                       num_devices=N_CORES)
        pt = nc.dram_tensor("pt", [D, B], BF16, kind="ExternalInput").ap()
        pr = nc.dram_tensor("pr", [B, D], BF16, kind="ExternalInput").ap()
        pblkt = nc.dram_tensor("pblkt", [D, BLK], BF16, kind="ExternalInput").ap()
        pblk = nc.dram_tensor("pblk", [BLK, D], BF16, kind="ExternalInput").ap()
        lse_out = nc.dram_tensor("lse_out", [128, M_TILES], F32,
                                 kind="ExternalOutput").ap()
        with tile.TileContext(nc) as tc:
            _emit(tc, pt, pr, pblkt, pblk, lse_out)
        nc.finalize()
        _BUILT = nc
    return _BUILT


def run_on_hw(P, **spmd_kwargs):
    import jax.numpy as jnp

    nc = _build()
    P_b = np.asarray(jnp.asarray(P, dtype=jnp.bfloat16))
    PT_b = np.ascontiguousarray(P_b.T)
    in_maps = [
        {
            "pt": PT_b,
            "pr": P_b,
            "pblkt": np.ascontiguousarray(PT_b[:, c * BLK : (c + 1) * BLK]),
            "pblk": np.ascontiguousarray(P_b[c * BLK : (c + 1) * BLK]),
        }
        for c in range(N_CORES)
    ]
    return bass_utils.run_bass_kernel_spmd(
        nc, in_maps, core_ids=list(range(N_CORES)), **spmd_kwargs
    )


def kernel(embedding1, embedding2, projection1, projection2):
    import jax.numpy as jnp

    # embeddings are unused by the reference computation
    P = np.ascontiguousarray(
        np.concatenate([projection1, projection2], axis=0), dtype=np.float32
    )
    res = run_on_hw(P)
    # reassemble per-row lse: core c, tile column m, partition p ->
    # global row c*1024 + m*128 + p
    lse_rows = np.empty(B, np.float32)
    for c in range(N_CORES):
        arr = np.asarray(res.results[c]["lse_out"])  # [128, M_TILES]
        lse_rows[c * BLK : (c + 1) * BLK] = arr.T.reshape(-1)
    # Reference fp32 semantics: logp_ii = f32(-2e9 - lse_i) (== -2e9 for
    # any |lse| < 128), then loss = -mean(logp) with the platform's XLA
    # fp32 reduction -- reproduce it bit-for-bit.
    logp = (np.float32(-2.0e9) - lse_rows).astype(np.float32)
    loss = -jnp.mean(jnp.asarray(logp))
    return np.asarray(loss)


# revision 11
# speedup vs baseline: 1.2529x; 1.0334x over previous
"""Contrastive (SimCLR-style) loss on 8 Trainium2 NeuronCores.

Math (matches the reference exactly):
  P = concat(projection1, projection2)            # [8192, 256]
  sim = cos_sim(P_i, P_j); diag masked to -1e9; logits = sim / 0.5
  labels = arange(2B)  -> picks the masked diagonal, so
  loss = -mean_i( logp_ii ),  logp_ii = f32(-2e9 - lse_i),
  lse_i = log(sum_{j != i} exp(2*sim_ij))

Distribution: data-parallel over the 8192 rows; every core holds the full
bf16 P^T as the moving matmul operand and its own RAW 1024-column slice as
the stationary operand (no on-chip transpose).  Per core:
  - global column norms from a row-major bf16 copy (DVE square + free-dim
    reduce in u=16 interleaved layout, bf16 partials, Newton rsqrt),
  - Q^T = P^T * (1/n_j) via a DRAM-bounced partition broadcast (bf16, DVE),
  - the row-side 1/n_i never touches the operands: it rides the ScalarE
    activation's per-partition scale (exp(scale_i * psum), scale_i = 2/n_i),
  - matmul row-block x all 8192 columns (bf16, fp32 PSUM, 2048-col groups),
  - exp streamed through ScalarE with fused row-sum accumulation,
  - rowsum - e^2 (diagonal of the normalized Gram is cos=1 -> exp(2)), log.

Scheduling: the tile scheduler is greedy per engine, so all prep that is
not on the group-0 critical chain is fenced behind tile_wait_until marks;
bulk DMAs are split across queues and trigger engines; the ScalarE Exp
table is preloaded by a dummy activation at t=0.
Host applies the reference's fp32 arithmetic for the final mean.
"""

import sys

for _p in ("/opt/trn_rl_repo", "/root/.axon_site/_ro/trn_rl_repo"):
    if _p not in sys.path:
        sys.path.append(_p)

import numpy as np

import concourse.bacc as bacc
import concourse.tile as tile
from concourse import mybir
from concourse import bass_utils

F32 = mybir.dt.float32
BF16 = mybir.dt.bfloat16
I32 = mybir.dt.int32
AF = mybir.ActivationFunctionType
ALU = mybir.AluOpType

N_CORES = 8
B = 8192          # total rows (2 * batch)
D = 256           # projection dim
BLK = B // N_CORES        # 1024 rows per core
M_TILES = BLK // 128      # 8 row tiles per core
N_COLS = 512              # matmul free dim (one PSUM bank)
GROUP = 2048              # ACT exp batch (4 PSUM banks) = one column group
N_GROUPS = B // GROUP     # 4
N_PER_GROUP = GROUP // N_COLS  # 4
U = 16                    # consecutive rows per partition in stats loads
UO = BLK // 128           # 8: row tiles in the own-block stats load
RSQRT_MAGIC = 0x5F3759DF
E2 = 7.38905609893065     # exp(2): the masked diagonal's exp term


def _newton_rsqrt(nc, pool, out_rn, s, tag):
    """out_rn = 1/sqrt(s), entirely on VectorE (fp32).

    Quake-style bit seed + 2 Newton iterations (~5e-6 rel err).  Keeps
    ScalarE free for exp and avoids sqrt<->exp table reloads.
    """
    p, w = s.shape
    ibits = pool.tile([p, w], I32, name="ibits", tag=f"rsqi_{tag}", bufs=1)
    nc.vector.tensor_scalar(
        out=ibits, in0=s.bitcast(I32), scalar1=1, scalar2=None,
        op0=ALU.arith_shift_right,
    )
    nc.vector.tensor_scalar(
        out=ibits, in0=ibits, scalar1=-1, scalar2=RSQRT_MAGIC,
        op0=ALU.mult, op1=ALU.add,
    )
    y = ibits.bitcast(F32)
    t1 = pool.tile([p, w], F32, name="t1", tag=f"rsqt_{tag}", bufs=1)
    for _ in range(2):
        nc.vector.tensor_mul(t1, y, y)
        nc.vector.tensor_mul(t1, t1, s)
        nc.vector.tensor_scalar(
            out=t1, in0=t1, scalar1=-0.5, scalar2=1.5,
            op0=ALU.mult, op1=ALU.add,
        )
        nc.vector.tensor_mul(y, y, t1)
    nc.vector.tensor_copy(out_rn, y)


def _emit(tc, pt, pr, pblkt, pblk, lse_out):
    nc = tc.nc

    persist = tc.alloc_tile_pool(name="persist", bufs=1)
    pin = tc.alloc_tile_pool(name="pin", bufs=2)
    work = tc.alloc_tile_pool(name="work", bufs=2)
    dram = tc.alloc_tile_pool(name="dram", bufs=1, space="DRAM")
    epool = tc.alloc_tile_pool(name="epool", bufs=2)
    psum_pool = tc.alloc_tile_pool(name="psum", bufs=2, space="PSUM")

    # Persistent tensors
    ptk = [persist.tile([128, B], BF16, tag=f"pt{k}", name=f"pt{k}")
           for k in range(2)]
    qtk = [persist.tile([128, B], BF16, tag=f"qt{k}", name=f"qt{k}")
           for k in range(2)]
    pblkt_t = [persist.tile([128, BLK], BF16, tag=f"pbt{k}", name=f"pbt{k}")
               for k in range(2)]
    rn_f = persist.tile([128, 64], F32, tag="rn_f", name="rn_f")
    rn_b16 = persist.tile([128, 64], BF16, tag="rn_b16", name="rn_b16")
    nblk = persist.tile([128, UO], BF16, tag="nblk", name="nblk")
    nblk_f = persist.tile([128, UO], F32, tag="nblk_f", name="nblk_f")
    scv = persist.tile([128, UO], F32, tag="scv", name="scv")
    sums = persist.tile([128, N_GROUPS * M_TILES], F32, tag="sums", name="sums")
    rowsum = persist.tile([128, M_TILES], F32, tag="rowsum", name="rowsum")
    lse = persist.tile([128, M_TILES], F32, tag="lse", name="lse")
    warm = persist.tile([1, 2], F32, tag="warm", name="warm")
    dram_rn = dram.tile([B], BF16, tag="dram_rn", name="dram_rn")

    # Preload the ScalarE Exp table off the critical path.
    nc.gpsimd.memset(warm, 0.0)
    nc.scalar.activation(out=warm[:, 1:2], in_=warm[:, 0:1], func=AF.Exp)

    # ---- Bulk loads, split across queues + trigger engines ----
    # pt: 8 chunks per k-half; gpsimd and tensor engines alternate triggers.
    for k in range(2):
        for h in range(8):
            eng = nc.gpsimd if (h % 2 == 0) else nc.scalar
            eng.dma_start(
                out=ptk[k][:, h * 1024 : (h + 1) * 1024],
                in_=pt[k * 128 : (k + 1) * 128, h * 1024 : (h + 1) * 1024],
            )
    for k in range(2):
        for h in range(2):
            nc.scalar.dma_start(
                out=pblkt_t[k][:, h * 512 : (h + 1) * 512],
                in_=pblkt[k * 128 : (k + 1) * 128, h * 512 : (h + 1) * 512],
            )

    # ---- Global column norms, one 2048-column group at a time.
    # Row-major interleave: row j = 2048g + 16p + u -> partition p, slot u.
    # ----
    ps4 = pr.rearrange("(g p u) d -> g p (u d)", p=128, u=U)   # [4,128,4096]
    rn_store = dram_rn.rearrange("(g p u) -> g p u", p=128, u=U)

    def normalize_group(g):
        gsl = slice(g * U, (g + 1) * U)
        pst = pin.tile([128, U * D], BF16, name="pst", tag="pst", bufs=2)
        for h in range(4):
            nc.sync.dma_start(
                out=pst[:, h * 1024 : (h + 1) * 1024],
                in_=ps4[g][:, h * 1024 : (h + 1) * 1024],
            )
        sq = work.tile([128, U * D], BF16, name="sq", tag="sq", bufs=2)
        nc.vector.tensor_mul(sq, pst, pst)
        nsq_b = work.tile([128, U], BF16, name="nsq_b", tag="nsq_b", bufs=2)
        with nc.allow_low_precision(reason="bf16 norm^2 partials, 0.4% ok"):
            nc.vector.tensor_reduce(
                nsq_b, sq.rearrange("p (u d) -> p u d", u=U),
                axis=mybir.AxisListType.X, op=ALU.add,
            )
        nc.vector.tensor_copy(rn_f[:, gsl], nsq_b)
        _newton_rsqrt(nc, work, rn_f[:, gsl], rn_f[:, gsl], tag=f"g{g}")
        nc.vector.tensor_copy(rn_b16[:, gsl], rn_f[:, gsl])
        nc.sync.dma_start(out=rn_store[g], in_=rn_b16[:, gsl])
        rnb = work.tile([128, GROUP], BF16, name="rnb", tag="rnb", bufs=2)
        for h in range(2):
            nc.sync.dma_start(
                out=rnb[:, h * 1024 : (h + 1) * 1024],
                in_=dram_rn[
                    g * GROUP + h * 1024 : g * GROUP + (h + 1) * 1024
                ].partition_broadcast(128),
            )
        for k in range(2):
            for h in range(2):
                hsl = slice(g * GROUP + h * 1024, g * GROUP + (h + 1) * 1024)
                nc.vector.tensor_mul(
                    qtk[k][:, hsl],
                    ptk[k][:, hsl],
                    rnb[:, h * 1024 : (h + 1) * 1024],
                )

    normalize_group(0)

    # ---- Own-block row norms -> per-partition ACT scale 2/n_i.
    # Row i = 128u + p -> partition p, slot u, so scv[:, m] lines up with
    # the psum partition dim of row-tile m.  First needed by the first EXP
    # (not the first matmul), so it is fenced after the group-0 chain. ----
    with tc.tile_wait_until(0.011):
        pbo = pblk.rearrange("(u p) d -> p u d", p=128, u=UO)
        pblk_il = pin.tile([128, UO * D], BF16, name="pblk_il",
                           tag="pblk_il", bufs=1)
        for h in range(2):
            nc.sync.dma_start(
                out=pblk_il.rearrange("p (u d) -> p u d", u=UO)[:, h * 4 : (h + 1) * 4, :],
                in_=pbo[:, h * 4 : (h + 1) * 4, :],
            )
        sq_o = work.tile([128, UO * D], BF16, name="sq_o", tag="sq_o", bufs=1)
        nc.vector.tensor_mul(sq_o, pblk_il, pblk_il)
        with nc.allow_low_precision(reason="bf16 norm^2 partials, 0.4% ok"):
            nc.vector.tensor_reduce(
                nblk, sq_o.rearrange("p (u d) -> p u d", u=UO),
                axis=mybir.AxisListType.X, op=ALU.add,
            )
        nc.vector.tensor_copy(nblk_f, nblk)
        _newton_rsqrt(nc, work, nblk_f, nblk_f, tag="own")
        nc.vector.tensor_scalar(
            out=scv, in0=nblk_f, scalar1=2.0, scalar2=None, op0=ALU.mult,
        )

    # ---- Main loop: S-block matmuls + fused exp/row-sum.  Group g+1 norm
    # prep is emitted after group g's body and fenced so the greedy
    # scheduler cannot wedge it into the group-0 critical chain. ----
    for g in range(N_GROUPS):
        for m in range(M_TILES):
            ps = psum_pool.tile([128, GROUP], F32, name="ps")
            for n4 in range(N_PER_GROUP):
                col = g * GROUP + n4 * N_COLS
                for k in range(2):
                    nc.tensor.matmul(
                        ps[:, n4 * N_COLS : (n4 + 1) * N_COLS],
                        pblkt_t[k][:, m * 128 : (m + 1) * 128],
                        qtk[k][:, col : col + N_COLS],
                        start=(k == 0),
                        stop=(k == 1),
                    )
            esc = epool.tile([128, GROUP], BF16, name="esc")
            nc.scalar.activation(
                out=esc,
                in_=ps,
                func=AF.Exp,
                scale=scv[:, m : m + 1],
                accum_out=sums[:, g * M_TILES + m : g * M_TILES + m + 1],
            )
        if g + 1 < N_GROUPS:
            with tc.tile_wait_until(0.014 + 0.014 * g):
                normalize_group(g + 1)

    # ---- Epilogue: rowsum over groups, drop diagonal, log, write out ----
    sums3 = sums.rearrange("p (g m) -> p m g", g=N_GROUPS)
    nc.vector.tensor_reduce(rowsum, sums3, axis=mybir.AxisListType.X, op=ALU.add)
    nc.vector.tensor_scalar(
        out=lse, in0=rowsum, scalar1=E2, scalar2=None, op0=ALU.subtract,
    )
    nc.scalar.activation(out=lse, in_=lse, func=AF.Ln)
    nc.sync.dma_start(out=lse_out, in_=lse)

    for p in (psum_pool, epool, dram, work, pin, persist):
        p.release()


_BUILT = None


def _build():
    global _BUILT
    if _BUILT is None:
        nc = bacc.Bacc("TRN2", target_bir_lowering=False, debug=False,
                       num_devices=N_CORES)
        pt = nc.dram_tensor("pt", [D, B], BF16, kind="ExternalInput").ap()
        pr = nc.dram_tensor("pr", [B, D], BF16, kind="ExternalInput").ap()
        pblkt = nc.dram_tensor("pblkt", [D, BLK], BF16, kind="ExternalInput").ap()
        pblk = nc.dram_tensor("pblk", [BLK, D], BF16, kind="ExternalInput").ap()
        lse_out = nc.dram_tensor("lse_out", [128, M_TILES], F32,
                                 kind="ExternalOutput").ap()
        with tile.TileContext(nc) as tc:
            _emit(tc, pt, pr, pblkt, pblk, lse_out)
        nc.finalize()
        _BUILT = nc
    return _BUILT


def run_on_hw(P, **spmd_kwargs):
    import jax.numpy as jnp

    nc = _build()
    P_b = np.asarray(jnp.asarray(P, dtype=jnp.bfloat16))
    PT_b = np.ascontiguousarray(P_b.T)
    in_maps = [
        {
            "pt": PT_b,
            "pr": P_b,
            "pblkt": np.ascontiguousarray(PT_b[:, c * BLK : (c + 1) * BLK]),
            "pblk": np.ascontiguousarray(P_b[c * BLK : (c + 1) * BLK]),
        }
        for c in range(N_CORES)
    ]
    return bass_utils.run_bass_kernel_spmd(
        nc, in_maps, core_ids=list(range(N_CORES)), **spmd_kwargs
    )


def kernel(embedding1, embedding2, projection1, projection2):
    import jax.numpy as jnp

    # embeddings are unused by the reference computation
    P = np.ascontiguousarray(
        np.concatenate([projection1, projection2], axis=0), dtype=np.float32
    )
    res = run_on_hw(P)
    # reassemble per-row lse: core c, tile column m, partition p ->
    # global row c*1024 + m*128 + p
    lse_rows = np.empty(B, np.float32)
    for c in range(N_CORES):
        arr = np.asarray(res.results[c]["lse_out"])  # [128, M_TILES]
        lse_rows[c * BLK : (c + 1) * BLK] = arr.T.reshape(-1)
    # Reference fp32 semantics: logp_ii = f32(-2e9 - lse_i) (== -2e9 for
    # any |lse| < 128), then loss = -mean(logp) with the platform's XLA
    # fp32 reduction -- reproduce it bit-for-bit.
    logp = (np.float32(-2.0e9) - lse_rows).astype(np.float32)
    loss = -jnp.mean(jnp.asarray(logp))
    return np.asarray(loss)
